# revision 12
# baseline (speedup 1.0000x reference)
"""Multi-head cross-attention TRN2 kernel (v2).

N=4096, D=256, H=4, K=16. Data-parallel over 8 NeuronCores: each core owns
512 query rows; key_value + weights replicated. No collectives.

Design notes (per core, R=512 query rows):
  - S^T form: scores chunk S^T[128 keys, 512 q] per (head, key-chunk).
  - Scores on PE in 64x128 row-tiled mode: heads packed 2 per 64-partition
    group (head h k-rows at partitions 32h..32h+16). One LDW of
    khT[64g:64g+64, chunk] serves both heads of group g; the two groups run
    concurrently on row tiles (0,0)/(64,0). Head h is selected by a
    zero-padded query operand (qt_A has heads 0,2; qt_B has heads 1,3).
  - PSUM: 3-slot ring of [128,1024] (2 banks each; slot = 2 heads x 512q)
    + 1 bank AV accumulator + 1 bank util (rb / tail).
  - Evacuation+exp of the 8.4M-element score tensor is the wall. It is
    split between ScalarE (native Exp activation) and VectorE (Schraudolph
    exp: tensor_scalar fp32->int16 (s*a+b) whose int16 bits are the bf16
    representation of ~exp(s); bitcast to bf16). Assignment alternates by
    chunk parity so each (head, query) row gets a 50/50 key mix of
    exact/approx exp (keeps the approx error averaged down).
  - AV on PE in 128x32 col-tiled mode: 4 heads concurrent, accumulating
    into one PSUM bank at partition bases 32h (rows 32h=denominator from
    the ones column, 32h+1..17 = the 16 V dims). A zeroing matmul opens the
    accumulation group so pad rows are 0.
  - Tail: strided-partition reciprocal of the 4 denominator rows, recip
    broadcast via indicator matmul, normalize, W_o matmul, DMA out.

Everything on the PE is bf16 (host-cast); accumulation fp32; output fp32.
"""
import numpy as np
import ml_dtypes

import concourse.bass as bass
from concourse import bacc
import concourse.mybir as mybir
import concourse.tile as tile
from concourse.bass_utils import run_bass_kernel_spmd

N, D, H, K = 4096, 256, 4, 16
NCORES = 8
R = N // NCORES          # 512 query rows per core
G = K + 1                # 17: ones column + 16 V dims per head
CH = 128                 # keys per chunk
NCH = N // CH            # 32 chunks
F32 = mybir.dt.float32
BF16 = mybir.dt.bfloat16
I16 = mybir.dt.int16
EXPF = mybir.ActivationFunctionType.Exp
BF = ml_dtypes.bfloat16

# Schraudolph exp for bf16 bits: exp(x) ~= bitcast_bf16(int16(x*SA + SB))
# bf16 exponent at bit 7; exp(x)=2^(x*log2e); geometric centering of the
# 2^f vs (1+f) mantissa error band (max ratio 1.0861) -> -7.62 bias.
# Scores need exp(S*0.25): fold 0.25 into the scale.
SCH_SCALE = 0.25 * 1.4426950408889634 * 128.0
SCH_BIAS = 127.0 * 128.0 - 7.62

TRACE = False
LAST_RESULTS = None

# weight blob geometry: per d-chunk dc (2 chunks of 128 d-rows):
#   cols 328*dc+0:128    wq (head h at cols 32h..32h+16)
#   cols 328*dc+128:256  wk (head h at cols 32h)
#   cols 328*dc+256:324  wv (head h at cols 17h+1..17h+17; col 17h zero)
WBLOB = 328  # 324 used + 4 pad (DMA transpose needs rows % 16 == 0)


def _build(repeats=1, dve_share=True):
    nc = bacc.Bacc()
    q = nc.declare_dram_parameter("q", [R, D], BF16, isOutput=False)
    kv = nc.declare_dram_parameter("kv", [N, D], BF16, isOutput=False)
    wqkv = nc.declare_dram_parameter("wqkv", [2 * WBLOB, 128], BF16, isOutput=False)
    wo = nc.declare_dram_parameter("wo", [128, D], BF16, isOutput=False)
    sel_d = nc.declare_dram_parameter("sel", [128, 128], F32, isOutput=False)
    out = nc.declare_dram_parameter("out", [R, D], F32, isOutput=True)

    with tile.TileContext(nc) as tc:
        with (
            tc.tile_pool(name="consts", bufs=1) as consts,
            tc.tile_pool(name="esa", bufs=3) as esapool,
            tc.tile_pool(name="esd", bufs=3) as esdpool,
            tc.tile_pool(name="sbops", bufs=2) as sbops,
            tc.tile_pool(name="sring", bufs=3, space="PSUM") as sring,
            tc.tile_pool(name="accp", bufs=1, space="PSUM") as accp,
            tc.tile_pool(name="utilp", bufs=1, space="PSUM") as utilp,
        ):
            for _rep in range(repeats):
                # ---- weights into SBUF ----
                wqkv_sb = consts.tile([128, 2 * WBLOB], BF16, tag="wqkv_sb",
                                      name="wqkv_sb")
                nc.sync.dma_start(out=wqkv_sb, in_=wqkv[:, :], transpose=True)
                wo_sb = consts.tile([128, D], BF16, tag="wo_sb", name="wo_sb")
                nc.sync.dma_start(out=wo_sb, in_=wo[:, :])

                def wq(dc):
                    return wqkv_sb[:, WBLOB * dc:WBLOB * dc + 128]

                def wk(dc):
                    return wqkv_sb[:, WBLOB * dc + 128:WBLOB * dc + 256]

                def wv(dc):
                    return wqkv_sb[:, WBLOB * dc + 256:WBLOB * dc + 324]

                # ---- transposed activations via DMA transpose ----
                qtr = [consts.tile([128, R], BF16, tag=f"qtr{i}", name=f"qtr{i}")
                       for i in range(2)]
                nc.sync.dma_start(out=qtr[0], in_=q[:, 0:128], transpose=True)
                nc.sync.dma_start(out=qtr[1], in_=q[:, 128:256], transpose=True)
                ktr = [consts.tile([128, N], BF16, tag=f"ktr{i}", name=f"ktr{i}")
                       for i in range(2)]
                for j in range(N // 512):
                    sl = slice(512 * j, 512 * (j + 1))
                    nc.sync.dma_start(out=ktr[0][:, sl], in_=kv[sl, 0:128],
                                      transpose=True)
                    nc.sync.dma_start(out=ktr[1][:, sl], in_=kv[sl, 128:256],
                                      transpose=True)

                # constant zero operands for the acc-zeroing matmul
                zcol = consts.tile([64, 128], BF16, tag="zcol", name="zcol")
                nc.vector.memset(zcol, 0.0)
                zrow = consts.tile([64, R], BF16, tag="zrow", name="zrow")
                nc.vector.memset(zrow, 0.0)
                # selector for denom broadcast: sel[32h, 32h:32h+32] = 1
                sel = consts.tile([128, 128], F32, tag="sel", name="sel")
                nc.sync.dma_start(out=sel, in_=sel_d[:, :])

                # ---- projections ----
                # qt [128 (head h k-rows at 32h), 512]
                qp = sring.tile([128, 1024], F32, tag="s", name="s")
                for dc in range(2):
                    nc.tensor.matmul(qp[:, 0:512], wq(dc), qtr[dc][:, :],
                                     start=(dc == 0), stop=(dc == 1))
                qt_sb = consts.tile([128, R], BF16, tag="qt_sb", name="qt_sb")
                nc.vector.tensor_copy(qt_sb[:], qp[:, 0:512])

                # khT [128 (head h k-rows at 32h), N] bf16
                khT = consts.tile([128, N], BF16, tag="khT", name="khT")
                for c4 in range(4):
                    kp = sring.tile([128, 1024], F32, tag="s", name="s")
                    for kb in range(2):
                        for dc in range(2):
                            ks = slice(1024 * c4 + 512 * kb,
                                       1024 * c4 + 512 * kb + 512)
                            nc.tensor.matmul(kp[:, 512 * kb:512 * kb + 512],
                                             wk(dc), ktr[dc][:, ks],
                                             start=(dc == 0), stop=(dc == 1))
                    dst = khT[:, 1024 * c4:1024 * (c4 + 1)]
                    if c4 % 2 == 0:
                        nc.scalar.copy(dst, kp[:])
                    else:
                        nc.vector.tensor_copy(dst, kp[:])

                # v_aug [128, 32*68]: chunk c cols 68c..68c+68; within: head h
                # at 17h (ones col) + 1..17 (V dims)
                v_aug = consts.tile([128, NCH * 4 * G], BF16, tag="v_aug",
                                    name="v_aug")
                v_ones = v_aug[:].rearrange("p (c h s) -> p c h s", h=H,
                                            s=G)[:, :, :, 0:1]
                nc.vector.memset(v_ones, 1.0)
                for vb in range(8):  # 4 chunks per slot
                    vp = sring.tile([128, 1024], F32, tag="s", name="s")
                    for j in range(4):
                        ck = 4 * vb + j
                        for dc in range(2):
                            nc.tensor.matmul(
                                vp[:, 68 * j:68 * (j + 1)],
                                ktr[dc][:, 128 * ck:128 * (ck + 1)],
                                wv(dc), start=(dc == 0), stop=(dc == 1))
                    vsrc = vp[:, 0:272].rearrange("p (j h s) -> p j h s",
                                                  h=H, s=G)[:, :, :, 1:G]
                    vdst = v_aug[:, 68 * 4 * vb:68 * 4 * (vb + 1)].rearrange(
                        "p (j h s) -> p j h s", h=H, s=G)[:, :, :, 1:G]
                    if vb % 2 == 0:
                        nc.vector.tensor_copy(vdst, vsrc)
                    else:
                        nc.scalar.copy(vdst, vsrc)

                # ---- main loop ----
                acc = accp.tile([128, R], F32, tag="acc", name="acc")
                # zero the acc bank (pad rows must be 0; opens the accum group)
                nc.tensor.matmul(acc[:], zcol[:], zrow[:], start=True,
                                 stop=False, skip_group_check=True)

                es_of = {}  # chunk -> (tile, col_base_is_h01)

                def scores(c):
                    lo = sring.tile([128, 1024], F32, tag="s", name="s")  # h0,h1
                    hi = sring.tile([128, 1024], F32, tag="s", name="s")  # h2,h3
                    for h in range(H):
                        slot = lo if h < 2 else hi
                        col = 512 * (h % 2)
                        nc.tensor.matmul(slot[:, col:col + 512],
                                         khT[32 * h:32 * h + 32,
                                             128 * c:128 * (c + 1)],
                                         qt_sb[32 * h:32 * h + 32, :],
                                         start=True, stop=True,
                                         tile_position=(32 * h, 0))
                    # evacuate + exp; alternate engines by parity, with 8
                    # chunks where DVE takes both slots (engine balance)
                    def exp_act(slot):
                        es = esapool.tile([128, 1024], BF16, tag="esa",
                                          name="esa")
                        nc.scalar.activation(es[:], slot[:], EXPF, scale=0.25)
                        return es

                    def exp_dve(slot):
                        es_i16 = esdpool.tile([128, 1024], I16, tag="esd",
                                              name="esd")
                        nc.vector.tensor_scalar(
                            es_i16[:], slot[:], SCH_SCALE, SCH_BIAS,
                            mybir.AluOpType.mult, mybir.AluOpType.add)
                        return es_i16.bitcast(BF16)

                    if c % 2 == 0:
                        es_of[c] = (exp_act(lo), exp_dve(hi), True)
                    else:
                        es_of[c] = (exp_act(hi), exp_dve(lo), False)

                def av(c):
                    es_act, es_dve, act_is_h01 = es_of.pop(c)
                    for h in range(H):
                        src = es_act if (h < 2) == act_is_h01 else es_dve
                        esl = src[:, 512 * (h % 2):512 * (h % 2) + 512]
                        nc.tensor.matmul(
                            acc[32 * h:32 * h + G, :],
                            v_aug[:, 68 * c + G * h:68 * c + G * (h + 1)],
                            esl, start=False, stop=(c == NCH - 1 and h == H - 1),
                            tile_position=(0, 32 * h), skip_group_check=True)

                BATCH = 8
                for b in range(NCH // BATCH):
                    for c in range(BATCH * b, BATCH * (b + 1)):
                        scores(c)
                    if b > 0:
                        for c in range(BATCH * (b - 1), BATCH * b):
                            av(c)
                for c in range(NCH - BATCH, NCH):
                    av(c)

                # ---- tail: normalize + W_o ----
                acc_sb = sbops.tile([128, R], F32, tag="acc_sb", name="acc_sb")
                nc.scalar.copy(acc_sb[:], acc[:])
                # rb = per-row broadcast of the head's denominator (fp32 mm)
                rb = utilp.tile([128, R], F32, tag="rb", name="rb")
                nc.tensor.matmul(rb[:], sel[:], acc_sb[:], start=True,
                                 stop=True)
                rbr = sbops.tile([128, R], BF16, tag="rbr", name="rbr")
                with nc.allow_low_precision(reason="bf16 recip feeds bf16 mm"):
                    nc.vector.reciprocal(rbr[:], rb[:])
                hn = sbops.tile([128, R], BF16, tag="hn", name="hn")
                nc.vector.tensor_mul(hn[:], acc_sb[:], rbr[:])
                wop = sring.tile([128, 1024], F32, tag="s", name="s")
                for qc in range(4):
                    nc.tensor.matmul(wop[:, 256 * qc:256 * (qc + 1)],
                                     hn[:, 128 * qc:128 * (qc + 1)],
                                     wo_sb[:, :], start=True, stop=True)
                out_sb = sbops.tile([128, 1024], F32, tag="out_sb",
                                    name="out_sb")
                nc.scalar.copy(out_sb[:, 0:512], wop[:, 0:512])
                nc.vector.tensor_copy(out_sb[:, 512:1024], wop[:, 512:1024])
                for qc in range(4):
                    nc.sync.dma_start(
                        out=out[128 * qc:128 * (qc + 1), :],
                        in_=out_sb[:, 256 * qc:256 * (qc + 1)])

    nc.finalize()
    return nc


_NC_CACHE = None


def _host_in_maps(query, key_value, W_q, W_k, W_v, W_o):
    q_bf = np.ascontiguousarray(query.astype(BF))
    kv_bf = np.ascontiguousarray(key_value.astype(BF))
    # blob [2*WBLOB, 128]: row 452*dc + c = column c of the per-d-chunk
    # weight block (see WBLOB comment); DMA transpose puts it at
    # wqkv_sb[:, 452*dc + c].
    wqt = np.transpose(W_q, (1, 0, 2))  # [D, H, K]
    wkt = np.transpose(W_k, (1, 0, 2))
    wvt = np.transpose(W_v, (1, 0, 2))
    blk = np.zeros((D, WBLOB), dtype=np.float32)
    for h in range(H):
        cq = 32 * h
        blk[:, cq:cq + K] = wqt[:, h, :]
        blk[:, 128 + cq:128 + cq + K] = wkt[:, h, :]
        blk[:, 256 + G * h + 1:256 + G * (h + 1)] = wvt[:, h, :]
    blob = np.concatenate([blk[0:128].T, blk[128:256].T], axis=0).astype(BF)
    blob = np.ascontiguousarray(blob)
    # wo blob [128, D]: row 32h+1+k = W_o[16h+k, :]; other rows zero
    wo_h = np.zeros((128, D), dtype=BF)
    wo_r = W_o.reshape(H, K, D)
    for h in range(H):
        wo_h[32 * h + 1:32 * h + 1 + K, :] = wo_r[h].astype(BF)
    sel = np.zeros((128, 128), dtype=np.float32)
    for h in range(H):
        sel[32 * h, 32 * h:32 * h + 32] = 1.0
    return [{"q": q_bf[c * R:(c + 1) * R], "kv": kv_bf, "wqkv": blob,
             "wo": wo_h, "sel": sel} for c in range(NCORES)]


def kernel(query, key_value, W_q, W_k, W_v, W_o):
    global _NC_CACHE, LAST_RESULTS
    if _NC_CACHE is None:
        _NC_CACHE = _build()
    nc = _NC_CACHE
    in_maps = _host_in_maps(query, key_value, W_q, W_k, W_v, W_o)
    res = run_bass_kernel_spmd(nc, in_maps, list(range(NCORES)), trace=TRACE)
    LAST_RESULTS = res
    return np.concatenate([res.results[c]["out"] for c in range(NCORES)], axis=0)
